# revision 31
# baseline (speedup 1.0000x reference)
"""Trainium2 Bass kernel for the CgpHmm scaled-forward layer.

Computes loglik[b] = scaled HMM forward log-likelihood over B=128 sequences
of length T=8192 with S=128 hidden states and an alphabet of E=6 symbols.

Strategy: data-parallel over batch (16 seqs/core on 8 cores) PLUS
speculative time-segmentation to break the sequential scan (segments
start from a uniform vector and re-run the last W=3 steps of the previous
segment as warmup; per-sequence loglik telescopes into sums of
log(colsum) captured at warmup end and segment end, assembled on host in
f64; emissions pre-divided by f_sym so no on-device renormalization).

ENGINE LAYOUT.  The emission multiply is the bottleneck resource; only
DVE can read PSUM for tensor*tensor; Act cannot multiply tensors;
gpsimd (Pool) multiplies SBUF-only at ~2ns/col.  Every recurrence lane
runs at cadence >= 2 so its serial path tolerates un-ramped PE:

  - 4 DVE chains (368 cols, 23 segs x 55 steps): chains 0,1 advance on
    even links, 2,3 on odd.  Per link: 2 matmuls + 2 DVE tensor_muls
    (PSUM f32 x em fp8 -> bf16) = ~1.06us busy = the period.
  - 4 pool tracks (<=512 cols): track j advances at links l%4==j through
    mm (PE) -> copy PSUM->SBUF bf16 (Act) -> tensor_mul (Pool).  ~2.4us
    path in a ~4.4us budget.  Per link: 1 mm + 1 Act copy + 1 Pool mul.
  - GLOBAL PSUM ROTATION: all matmuls (chains and tracks) write the
    next of 8 one-bank [P,512] PSUM slots, round robin.  The WAR edge
    mm[n+8] <- multiply[n] bounds lane drift to ~2.7 links: without it
    the dataflow scheduler let fast lanes run ~35 links ahead and the
    straggler chains drained latency-bound at the end (measured 183us
    with a 36us two-chain tail; with rotation: 142.5us).
  - Outputs bf16: stash (warmup captures) Act-copied into one packed
    SBUF tile, DMA'd once mid-program; track finals DMA'd as each track
    ends (hidden under remaining links); chain finals DMA'd from the
    last alpha ring slots.  alpha is bf16 anyway, f32 staging added no
    precision.
  - Preamble: init alpha is uniform 1/P everywhere except chain0's 16
    seg-0 columns, so it is memset on the (idle) gpsimd engine + one
    4KB DMA rather than loaded (889KB); each dma_start costs ~0.6us of
    serial issue time on an HWDGE queue (SP/Act), so first em chunks
    are one slice per stream, split across both queues.

Coverage per sequence (W=3): c-seg0 = steps 1..58, 91 more c-segs x 55,
tracks 32x26 + 32x25 + 32x25 + 29x24 = 3128: 58 + 5005 + 3128 = 8191.
116 links.  Measured 142.5us (baseline 195.4us): ~1.07us/link with DVE
98.7% / Pool 96% / PE 97% busy, + ~10us preamble + ~7us drain tail.
"""

import sys

import numpy as np

sys.path.insert(0, "/opt/trn_rl_repo")

P = 128          # states / partitions
BL = 16          # sequences per core
N_CORES = 8
B_FULL = 128
T_FULL = 8192
E_SYM = 6

W_WARM = 3
N_CH = 4                      # DVE chains, cadence 2
SEGS_C = 23                   # segments per DVE chain
L_C = 55                      # real steps per chain segment (seg0: +W)
CLINKS = L_C + W_WARM         # 58 chain links
LINKS = 2 * CLINKS            # 116 global links
W_C = SEGS_C * BL             # 368 cols per chain

TRACKS = [(32, 26), (32, 25), (32, 25), (29, 24)]   # (segs, real steps)
N_TRK = len(TRACKS)
W_T = [s * BL for s, _ in TRACKS]          # [512, 512, 512, 464]
TLINKS = [lt + W_WARM for _, lt in TRACKS]  # [29, 28, 28, 27]
for j, tl in enumerate(TLINKS):
    assert (tl - 1) * N_TRK + j < LINKS, "track overruns program"
assert all(w * 4 <= 2048 for w in W_T) and W_C * 4 <= 2048

COV_D = (L_C + W_WARM) + (N_CH * SEGS_C - 1) * L_C
COV_T = sum(s * lt for s, lt in TRACKS)
assert COV_D + COV_T == T_FULL - 1, (COV_D, COV_T)

TRK_T0 = []
_t = 1 + COV_D
for s, lt in TRACKS:
    TRK_T0.append(_t)
    _t += s * lt

N_ABUF = 3
CB_C = [0, 2]                 # em chunks for chains, in chain links
while CB_C[-1] < CLINKS:
    CB_C.append(min(CLINKS, CB_C[-1] + 8))
CB_T = [0, 2]                 # em chunks for tracks, in track links
while CB_T[-1] < max(TLINKS):
    CB_T.append(min(max(TLINKS), CB_T[-1] + 8))
N_DMA_SLICES = 2
MM_DTYPE = "bfloat16"
EM_DTYPE = "float8e4"

# packed init/stash layout: [chain0..3 | track0..3]
PACK_OFF = []
_o = 0
for w in [W_C] * N_CH + W_T:
    PACK_OFF.append(_o)
    _o += w
PACK_COLS = _o                # 3472


def _cseg_t0(s):
    """Warmup-start step of chain segment s (global index 0..91)."""
    if s == 0:
        return 1                            # covers steps 1..58, no warmup
    return 1 + CLINKS + (s - 1) * L_C - W_WARM


def _tseg_t0(j, p):
    """Warmup-start step of track j's segment p."""
    return TRK_T0[j] + p * TRACKS[j][1] - W_WARM


def build_nc(debug=False):
    import concourse.bacc as bacc
    import concourse.bass as bass  # noqa: F401
    import concourse.mybir as mybir
    import concourse.tile as tile

    nc = bacc.Bacc(None, target_bir_lowering=False, debug=debug)

    f32 = mybir.dt.float32
    mdt = getattr(mybir.dt, MM_DTYPE)
    edt = getattr(mybir.dt, EM_DTYPE)

    em_c = [nc.dram_tensor(f"emc{j}", [P, CLINKS * W_C], edt,
                           kind="ExternalInput") for j in range(N_CH)]
    em_t = [nc.dram_tensor(f"emt{j}", [P, TLINKS[j] * W_T[j]], edt,
                           kind="ExternalInput") for j in range(N_TRK)]
    a_d = nc.dram_tensor("amat", [P, P], mdt, kind="ExternalInput")
    ainit_d = nc.dram_tensor("ainit", [P, BL], mdt,
                             kind="ExternalInput")    # chain0 seg0 alpha0
    afc = [nc.dram_tensor(f"afinc{j}", [P, W_C], mdt,
                          kind="ExternalOutput") for j in range(N_CH)]
    aft = [nc.dram_tensor(f"afint{j}", [P, W_T[j]], mdt,
                          kind="ExternalOutput") for j in range(N_TRK)]
    stash_d = nc.dram_tensor("stash", [P, PACK_COLS], mdt,
                             kind="ExternalOutput")

    def chunk_dma(eng, emb_tile, em_dram, bounds, k, w, nlinks, nsl):
        l0 = bounds[k]
        l1 = min(bounds[k + 1], nlinks)
        if l0 >= l1:
            return
        cols_k = (l1 - l0) * w
        base = l0 * w
        per = (cols_k + nsl - 1) // nsl
        for s in range(nsl):
            o0 = s * per
            o1 = min(cols_k, o0 + per)
            if o0 >= o1:
                break
            eng.dma_start(emb_tile[:, o0:o1],
                          em_dram[:, base + o0:base + o1])

    cmax_c = max(b - a for a, b in zip(CB_C, CB_C[1:]))
    cmax_t = max(b - a for a, b in zip(CB_T, CB_T[1:]))

    with tile.TileContext(nc) as tc, \
            tc.tile_pool(name="sb", bufs=1) as sbp, \
            tc.tile_pool(name="ps", bufs=1, space="PSUM") as psp:
        a_sb = sbp.tile([P, P], mdt, name="a_sb")
        ainit_sb = sbp.tile([P, PACK_COLS], mdt, name="ainit_sb")
        # alpha rings: slot 0 is a slice of the packed init tile
        alc = [[ainit_sb[:, PACK_OFF[j]:PACK_OFF[j] + W_C]] +
               [sbp.tile([P, W_C], mdt, name=f"alc{j}_{k}")[:]
                for k in range(1, N_ABUF)] for j in range(N_CH)]
        alt = [[ainit_sb[:, PACK_OFF[N_CH + j]:
                         PACK_OFF[N_CH + j] + W_T[j]]] +
               [sbp.tile([P, W_T[j]], mdt, name=f"alt{j}_{k}")[:]
                for k in range(1, N_ABUF)] for j in range(N_TRK)]
        embc = [[sbp.tile([P, cmax_c * W_C], edt, name=f"embc{j}_{k}")
                 for k in range(2)] for j in range(N_CH)]
        embt = [[sbp.tile([P, cmax_t * W_T[j]], edt, name=f"embt{j}_{k}")
                 for k in range(2)] for j in range(N_TRK)]
        tmp = [sbp.tile([P, W_T[j]], mdt, name=f"tmp{j}")
               for j in range(N_TRK)]
        stash_sb = sbp.tile([P, PACK_COLS], mdt, name="stash_sb")
        # global PSUM rotation: 8 one-bank slots shared by ALL matmuls
        ps8 = [psp.tile([P, 512], f32, name=f"ps8_{k}") for k in range(8)]
        slot = [0]

        def next_ps(w):
            pp = ps8[slot[0] % 8][:, :w]
            slot[0] += 1
            return pp

        # preamble: init alpha is uniform 1/P except chain0's 16 seg-0
        # columns -- memset on (otherwise idle) gpsimd + one tiny DMA
        # instead of an 889KB load; em chunk0s split across both HWDGE
        # queues
        nc.sync.dma_start(a_sb[:], a_d[:])
        nc.gpsimd.memset(ainit_sb[:], 1.0 / P)
        nc.scalar.dma_start(ainit_sb[:, :BL], ainit_d[:])
        for j in range(N_CH):
            chunk_dma(nc.sync if j % 2 == 0 else nc.scalar,
                      embc[j][0], em_c[j], CB_C, 0, W_C, CLINKS, 1)
        for j in range(N_TRK):
            chunk_dma(nc.sync if j % 2 == 0 else nc.scalar,
                      embt[j][0], em_t[j], CB_T, 0, W_T[j], TLINKS[j], 1)

        # load A as the PE stationary operand (result discarded)
        nc.tensor.matmul(next_ps(W_C), a_sb[:], alc[0][0])
        slot[0] = 0

        import bisect
        for l in range(LINKS):
            cl = l // 2
            pair = (0, 1) if l % 2 == 0 else (2, 3)
            kc = bisect.bisect_right(CB_C, cl) - 1
            if cl == CB_C[kc] and l % 2 == 0 and kc + 1 < len(CB_C) - 1:
                for j in range(N_CH):
                    # sync queue only: a dma_start on the Act queue
                    # delays Act's track copies (measured +7.5us)
                    chunk_dma(nc.sync, embc[j][(kc + 1) % 2], em_c[j],
                              CB_C, kc + 1, W_C, CLINKS, N_DMA_SLICES)
            for j in pair:
                c0 = (cl - CB_C[kc]) * W_C
                cur = alc[j][cl % N_ABUF]
                nxt = alc[j][(cl + 1) % N_ABUF]
                pp = next_ps(W_C)
                nc.tensor.matmul(pp, a_sb[:], cur)
                nc.vector.tensor_mul(nxt, pp,
                                     embc[j][kc % 2][:, c0:c0 + W_C])
                if cl == W_WARM - 1:
                    nc.scalar.copy(
                        stash_sb[:, PACK_OFF[j]:PACK_OFF[j] + W_C], nxt)
            # pool track l%4
            j = l % N_TRK
            tl = l // N_TRK
            if tl < TLINKS[j]:
                kt = bisect.bisect_right(CB_T, tl) - 1
                if tl == CB_T[kt] and kt + 1 < len(CB_T) - 1:
                    chunk_dma(nc.sync, embt[j][(kt + 1) % 2], em_t[j],
                              CB_T, kt + 1, W_T[j], TLINKS[j],
                              N_DMA_SLICES)
                c0 = (tl - CB_T[kt]) * W_T[j]
                cur = alt[j][tl % N_ABUF]
                nxt = alt[j][(tl + 1) % N_ABUF]
                pp = next_ps(W_T[j])
                tm = tmp[j]
                nc.tensor.matmul(pp, a_sb[:], cur)
                nc.scalar.copy(tm[:], pp)
                nc.gpsimd.tensor_mul(nxt, tm[:],
                                     embt[j][kt % 2][:, c0:c0 + W_T[j]])
                if tl == W_WARM - 1:
                    off = PACK_OFF[N_CH + j]
                    nc.scalar.copy(stash_sb[:, off:off + W_T[j]], nxt)
                if tl == TLINKS[j] - 1:
                    # ship this track's final alpha now -- hidden under
                    # the remaining chain links (sync queue only; Act's
                    # queue must stay clear for its copies)
                    nc.sync.dma_start(aft[j][:], alt[j][TLINKS[j] % N_ABUF])
            if l == 24:
                # all stash captures done by link 9; ship them now
                nc.sync.dma_start(stash_d[:], stash_sb[:])

        # finals: DMA the last chain alpha ring slots directly (bf16),
        # two slices each across both HWDGE queues
        for j in range(N_CH):
            h = W_C // 2
            src = alc[j][CLINKS % N_ABUF]
            nc.sync.dma_start(afc[j][:, :h], src[:, :h])
            nc.scalar.dma_start(afc[j][:, h:], src[:, h:])

    # A never changes: strip all but the first ldweights so matmuls reuse
    # the resident PE array.
    seen_ldw = False
    for f in nc.m.functions:
        for b in f.blocks:
            new = []
            for ins in b.instructions:
                if isinstance(ins, mybir.InstLdweights):
                    si = ins.sync_info
                    has_sync = si is not None and (
                        len(si.on_wait or []) or len(si.on_update or []))
                    if seen_ldw and not has_sync:
                        continue
                    seen_ldw = True
                new.append(ins)
            b.instructions[:] = new

    nc.compile()
    return nc


def host_prepare(obs, I, A, Bm):
    """Shard + precompute per-core device inputs and host bookkeeping."""
    import ml_dtypes
    bf16 = ml_dtypes.bfloat16
    import concourse.mybir as mybir
    em_np = mybir.dt.np(getattr(mybir.dt, EM_DTYPE))

    obs = np.asarray(obs)
    I64 = np.asarray(I, np.float64)
    A64 = np.asarray(A, np.float64)
    Bm64 = np.asarray(Bm, np.float64)

    pi = np.full(P, 1.0 / P)
    for _ in range(300):
        pi = pi @ A64
    f_sym = pi @ Bm64                                   # [E]
    Bmh = (Bm64 / f_sym[None, :]).astype(np.float32)    # folded emissions
    Bmh_em = Bmh.astype(em_np)

    A_bf = np.asarray(A, np.float32).astype(bf16)

    t0c = np.array([_cseg_t0(s) for s in range(N_CH * SEGS_C)])
    stepc = t0c[:, None] + np.arange(CLINKS)[None, :]        # [92, 58]
    stept = []
    for j in range(N_TRK):
        t0t = np.array([_tseg_t0(j, p) for p in range(TRACKS[j][0])])
        stept.append(t0t[:, None] + np.arange(TLINKS[j])[None, :])

    in_maps = []
    book = []
    for c in range(N_CORES):
        ob = obs[c * BL:(c + 1) * BL]                   # [16, T]
        a0 = I64[:, None] * Bm64[:, ob[:, 0]]           # [S, 16]
        Z0 = a0.sum(0)
        alpha0 = (a0 / Z0).astype(np.float32).astype(bf16)

        m = {"amat": A_bf, "ainit": np.ascontiguousarray(alpha0)}
        for j in range(N_CH):
            gs = np.arange(j * SEGS_C, (j + 1) * SEGS_C)
            sym = ob[:, stepc[gs]]                      # [16, segs, CL]
            sym = sym.transpose(2, 1, 0).reshape(-1)
            m[f"emc{j}"] = np.ascontiguousarray(Bmh_em[:, sym])
        for j in range(N_TRK):
            sym = ob[:, stept[j]]                       # [16, segs, TL]
            sym = sym.transpose(2, 1, 0).reshape(-1)
            m[f"emt{j}"] = np.ascontiguousarray(Bmh_em[:, sym])
        in_maps.append(m)

        cnt = np.stack([(ob[:, 1:] == e).sum(1) for e in range(E_SYM)], 1)
        ll_base = np.log(Z0) + (cnt * np.log(f_sym)[None, :]).sum(1)  # [16]
        book.append(ll_base)
    return in_maps, book


def assemble_output(results, book):
    """Combine device outputs + host bookkeeping into loglik [128] f32."""
    out = np.empty(B_FULL, np.float64)
    for c in range(N_CORES):
        r = results[c]
        ll = book[c].copy()                             # [16]
        stash = r["stash"].astype(np.float64)           # [P, PACK_COLS]
        for j in range(N_CH):
            cs_e = r[f"afinc{j}"].astype(np.float64).reshape(
                P, SEGS_C, BL).sum(0)
            o = PACK_OFF[j]
            cs_w = stash[:, o:o + W_C].reshape(P, SEGS_C, BL).sum(0)
            ll += np.log(cs_e).sum(0)
            lw = np.log(cs_w)
            if j == 0:
                lw = lw[1:]                             # seg 0: no warmup
            ll -= lw.sum(0)
        for j in range(N_TRK):
            cs_e = r[f"afint{j}"].astype(np.float64).reshape(
                P, TRACKS[j][0], BL).sum(0)
            o = PACK_OFF[N_CH + j]
            cs_w = stash[:, o:o + W_T[j]].reshape(
                P, TRACKS[j][0], BL).sum(0)
            ll += np.log(cs_e).sum(0) - np.log(cs_w).sum(0)
        out[c * BL:(c + 1) * BL] = ll
    return out.astype(np.float32)


_NC_CACHE = {}


def _get_nc():
    if "nc" not in _NC_CACHE:
        _NC_CACHE["nc"] = build_nc()
    return _NC_CACHE["nc"]


def kernel(obs, I, A, Bm):
    from concourse.bass_utils import run_bass_kernel_spmd

    nc = _get_nc()
    in_maps, book = host_prepare(obs, I, A, Bm)
    res = run_bass_kernel_spmd(nc, in_maps, core_ids=list(range(N_CORES)))
    return assemble_output(res.results, book)
